# revision 1
# baseline (speedup 1.0000x reference)
"""Trainium2 Bass kernel for nn_DepthwiseXCorr (SiamRPN++-style depthwise-xcorr head).

Pipeline per sample (data-parallel over batch: 64 samples -> 8 cores x 8):
  conv3x3(kernel,wk)+BN+ReLU -> k_feat [256,5,5]
  conv3x3(search,ws)+BN+ReLU -> s_feat [256,29,29]
  depthwise xcorr(s_feat,k_feat) -> feat [256,25,25]
  1x1 conv w1 + BN + ReLU -> h [256,25,25]
  1x1 conv w2 + b2 -> out [20,25,25]

Matmuls run in float32r (TF32-class, ~1.6e-4 scale-rel error). The depthwise
xcorr is split between the PE (per-tap diagonal-weight matmuls) and the DVE
(scalar_tensor_tensor multiply-accumulate), balancing engine occupancy.
"""
import numpy as np

EPS = 1e-5
NCORES = 8
BPC = 8          # samples per core
NPE_SCHED = [9, 9, 10, 10, 10, 11, 14, 19]  # xcorr taps on PE per sample; rest on DVE

_CACHE = {}


def _shift_window(ap_2d, base_off, rows, cols, rowstride):
    """AP reading [128, rows, cols] window at element offset base_off of a
    [128, W] SBUF view, row stride in elements."""
    import concourse.bass as bass
    return bass.AP(
        tensor=ap_2d.tensor,
        offset=ap_2d.offset + base_off,
        ap=[list(ap_2d.ap[0]), [rowstride, rows], [1, cols]],
    )


def _build(bench_R=0):
    import concourse.bacc as bacc
    import concourse.mybir as mybir
    import concourse.tile as tile

    f32 = mybir.dt.float32
    f32r = mybir.dt.float32r
    AF = mybir.ActivationFunctionType
    ALU = mybir.AluOpType

    nc = bacc.Bacc("TRN2", target_bir_lowering=False, debug=False,
                   num_devices=NCORES)

    search_d = nc.declare_dram_parameter("search", [BPC, 128, 2, 968], f32r, isOutput=False)
    tmpl_d = nc.declare_dram_parameter("tmpl", [128, 2, BPC, 52], f32r, isOutput=False)
    wkt_d = nc.declare_dram_parameter("wkt", [128, 36, 128], f32r, isOutput=False)
    wst_d = nc.declare_dram_parameter("wst", [128, 36, 128], f32r, isOutput=False)
    w1t_d = nc.declare_dram_parameter("w1t", [128, 4, 128], f32r, isOutput=False)
    w2t_d = nc.declare_dram_parameter("w2t", [128, 2, 20], f32r, isOutput=False)
    bnk_d = nc.declare_dram_parameter("bnk", [128, 4], f32, isOutput=False)
    bns_d = nc.declare_dram_parameter("bns", [128, 4], f32, isOutput=False)
    bnh_d = nc.declare_dram_parameter("bnh", [128, 4], f32, isOutput=False)
    b2_d = nc.declare_dram_parameter("b2t", [128, 1], f32, isOutput=False)
    id_d = nc.declare_dram_parameter("ident", [128, 128], f32, isOutput=False)
    out_d = nc.declare_dram_parameter("out", [BPC, 20, 625], f32, isOutput=True)

    def tidx(cig, dy, dx, og):
        return ((cig * 3 + dy) * 3 + dx) * 2 + og

    with tile.TileContext(nc) as tc:
        with (
            tc.tile_pool(name="wp", bufs=1) as wp,
            tc.tile_pool(name="sp", bufs=2) as sp,
            tc.tile_pool(name="fp", bufs=2) as fp,
            tc.tile_pool(name="dp", bufs=16) as dp,
            tc.tile_pool(name="psc", bufs=4, space="PSUM") as psc,
            tc.tile_pool(name="psx", bufs=2, space="PSUM") as psx,
            tc.tile_pool(name="psh", bufs=2, space="PSUM") as psh,
        ):
            wkt = wp.tile([128, 36, 128], f32r)
            wst = wp.tile([128, 36, 128], f32r)
            w1t = wp.tile([128, 4, 128], f32r)
            w2t = wp.tile([128, 2, 20], f32r)
            bnk = wp.tile([128, 4], f32)
            bns = wp.tile([128, 4], f32)
            bnh = wp.tile([128, 4], f32)
            b2t = wp.tile([128, 1], f32)
            ident = wp.tile([128, 128], f32)
            k_in = wp.tile([128, 2, BPC, 52], f32r)
            nc.gpsimd.dma_start(out=k_in, in_=tmpl_d[:, :, :, :])
            nc.gpsimd.dma_start(out=bnk, in_=bnk_d[:, :])
            nc.sync.dma_start(out=wkt, in_=wkt_d[:, :, :])
            nc.sync.dma_start(out=wst, in_=wst_d[:, :, :])
            nc.sync.dma_start(out=w1t, in_=w1t_d[:, :, :])
            nc.sync.dma_start(out=w2t, in_=w2t_d[:, :, :])
            nc.gpsimd.dma_start(out=bns, in_=bns_d[:, :])
            nc.gpsimd.dma_start(out=bnh, in_=bnh_d[:, :])
            nc.gpsimd.dma_start(out=b2t, in_=b2_d[:, :])
            nc.gpsimd.dma_start(out=ident, in_=id_d[:, :])

            # conv_kernel branch: all samples batched, N = 8*35 = 280
            k_feat = wp.tile([128, 2, BPC * 36], f32)
            for og in range(2):
                pk = psc.tile([128, 512], f32, tag="conv")
                j = 0
                for cig in range(2):
                    for dy in range(3):
                        for dx in range(3):
                            rhs = k_in[:, cig, :, dy * 7 + dx: dy * 7 + dx + 36]
                            nc.tensor.matmul(pk[:, :288], wkt[:, tidx(cig, dy, dx, og), :],
                                             rhs, start=(j == 0), stop=(j == 17))
                            j += 1
                nc.scalar.activation(k_feat[:, og, :], pk[:, :288], AF.Relu,
                                     scale=bnk[:, og:og + 1], bias=bnk[:, 2 + og:3 + og])

            taps = [(t // 5, t % 5) for t in range(25)]

            import contextlib
            loop_cm = (tc.For_i(0, bench_R, 1,
                                  hint_engines=(mybir.EngineType.PE,
                                                mybir.EngineType.DVE,
                                                mybir.EngineType.Activation))
                         if bench_R else contextlib.nullcontext())
            with loop_cm:
              for s in range(BPC):
                  NPE = NPE_SCHED[s]
                  pe_taps, dve_taps = taps[:NPE], taps[NPE:]
                  s_in = sp.tile([128, 2, 968], f32r, tag="s_in")
                  nc.scalar.dma_start(out=s_in, in_=search_d[s, :, :, :])

                  # conv_search: out plane 29 rows x 30 cols (1 garbage col/row)
                  s_feat = sp.tile([128, 2, 870], f32r, tag="s_feat")
                  for og in range(2):
                      for off, y0c, rws in ((0, 0, 17), (510, 17, 12)):
                          w = rws * 30
                          pc = psc.tile([128, 512], f32, tag="conv")
                          j = 0
                          for cig in range(2):
                              for dy in range(3):
                                  for dx in range(3):
                                      rhs = _shift_window(s_in[:, cig, :], (y0c + dy) * 31 + dx,
                                                          rws, 30, 31)
                                      nc.tensor.matmul(pc[:, :w], wst[:, tidx(cig, dy, dx, og), :],
                                                       rhs, start=(j == 0), stop=(j == 17))
                                      j += 1
                          nc.scalar.activation(s_feat[:, og, off:off + w], pc[:, :w], AF.Relu,
                                               scale=bns[:, og:og + 1], bias=bns[:, 2 + og:3 + og])

                  # depthwise xcorr -> feat [128, 2, 625]
                  feat = fp.tile([128, 2, 625], f32, tag="feat")
                  featr = fp.tile([128, 2, 640], f32r, tag="featr")
                  nc.gpsimd.memset(featr[:, :, 625:640].bitcast(f32), 0.0)
                  for og in range(2):
                      sf = s_feat[:, og, :]
                      if NPE > 0:
                          dlist = []
                          for (dy, dx) in pe_taps:
                              diag = dp.tile([128, 128], f32r, tag="diag")
                              nc.scalar.activation(
                                  diag, ident, AF.Copy,
                                  scale=k_feat[:, og, s * 36 + dy * 7 + dx: s * 36 + dy * 7 + dx + 1])
                              dlist.append(diag)
                          for y0, rows in ((0, 13), (13, 12)):
                              px = psx.tile([128, 338], f32, tag="x")
                              n = rows * 26
                              for i, (dy, dx) in enumerate(pe_taps):
                                  rhs = _shift_window(sf, (y0 + dy) * 30 + dx, rows, 26, 30)
                                  nc.tensor.matmul(px[:, :n], dlist[i], rhs,
                                                   start=(i == 0), stop=(i == NPE - 1))
                              src_px = _shift_window(px, 0, rows, 25, 26)
                              dst_f = feat[:, og, y0 * 25: y0 * 25 + rows * 25].rearrange(
                                  "p (r c) -> p r c", c=25)
                              nc.scalar.activation(dst_f, src_px, AF.Copy)
                      fv = feat[:, og, 0:625].rearrange("p (r c) -> p r c", c=25)
                      frv = featr[:, og, 0:625].rearrange("p (r c) -> p r c", c=25)
                      nlast = len(dve_taps) - 1
                      for j, (dy, dx) in enumerate(dve_taps):
                          win = _shift_window(sf, dy * 30 + dx, 25, 25, 30)
                          kap = k_feat[:, og, s * 36 + dy * 7 + dx: s * 36 + dy * 7 + dx + 1]
                          dst = frv if j == nlast else fv
                          if NPE == 0 and j == 0:
                              nc.vector.tensor_scalar(dst, win, kap, None, ALU.mult)
                          else:
                              nc.vector.scalar_tensor_tensor(dst, win, kap, fv, ALU.mult, ALU.add)
                      if not dve_taps:
                          nc.scalar.activation(featr[:, og, 0:625], feat[:, og, :], AF.Copy)

                  # head: 1x1 conv -> BN -> ReLU -> 1x1 conv + b2
                  h = fp.tile([128, 2, 640], f32r, tag="h")
                  for og in range(2):
                      for off, w in ((0, 320), (320, 306)):
                          ph = psh.tile([128, 320], f32, tag="h")
                          nc.tensor.matmul(ph[:, :w], w1t[:, 0 * 2 + og, :],
                                           featr[:, 0, off:off + w], start=True, stop=False)
                          nc.tensor.matmul(ph[:, :w], w1t[:, 1 * 2 + og, :],
                                           featr[:, 1, off:off + w], start=False, stop=True)
                          nc.scalar.activation(h[:, og, off:off + w], ph[:, :w], AF.Relu,
                                               scale=bnh[:, og:og + 1], bias=bnh[:, 2 + og:3 + og])

                  out_s = fp.tile([128, 640], f32, tag="outs")
                  for off, w in ((0, 320), (320, 306)):
                      po = psh.tile([128, 320], f32, tag="h")
                      nc.tensor.matmul(po[0:20, :w], w2t[:, 0, :], h[:, 0, off:off + w],
                                       start=True, stop=False)
                      nc.tensor.matmul(po[0:20, :w], w2t[:, 1, :], h[:, 1, off:off + w],
                                       start=False, stop=True)
                      nc.scalar.activation(out_s[0:20, off:off + w], po[0:20, :w],
                                           AF.Identity, bias=b2t[0:20, 0:1])
                  nc.sync.dma_start(out=out_d[s, :, :], in_=out_s[0:20, 0:625])

    nc.compile()
    return nc


def _pack(inputs):
    f32 = np.float32
    kern = np.ascontiguousarray(inputs["kernel"], dtype=f32)
    search = np.ascontiguousarray(inputs["search"], dtype=f32)
    wk, ws = inputs["wk"].astype(f32), inputs["ws"].astype(f32)
    w1, w2, b2 = inputs["w1"].astype(f32), inputs["w2"].astype(f32), inputs["b2"].astype(f32)

    def fold(scale, bias, mean, var):
        inv = scale.astype(f32) / np.sqrt(var.astype(f32) + EPS)
        sh = bias.astype(f32) - mean.astype(f32) * inv
        arr = np.zeros((128, 4), f32)
        arr[:, 0:2] = inv.reshape(2, 128).T
        arr[:, 2:4] = sh.reshape(2, 128).T
        return arr

    bnk = fold(inputs["bnk_scale"], inputs["bnk_bias"], inputs["bnk_mean"], inputs["bnk_var"])
    bns = fold(inputs["bns_scale"], inputs["bns_bias"], inputs["bns_mean"], inputs["bns_var"])
    bnh = fold(inputs["bnh_scale"], inputs["bnh_bias"], inputs["bnh_mean"], inputs["bnh_var"])

    # conv weights -> lhsT tiles [ci, (cig,dy,dx,og), co]
    def conv_w(w):
        w6 = w.reshape(2, 128, 2, 128, 3, 3)           # og co cig ci dy dx
        return np.ascontiguousarray(w6.transpose(3, 2, 4, 5, 0, 1).reshape(128, 36, 128))

    wkt, wst = conv_w(wk), conv_w(ws)
    w1t = np.ascontiguousarray(
        w1[:, :, 0, 0].reshape(2, 128, 2, 128).transpose(3, 2, 0, 1).reshape(128, 4, 128))
    w2t = np.ascontiguousarray(
        w2[:, :, 0, 0].reshape(20, 2, 128).transpose(2, 1, 0))
    b2t = np.zeros((128, 1), f32)
    b2t[:20, 0] = b2
    ident = np.eye(128, dtype=f32)

    # search [64,256,31,31] -> per core [8, 128(ci), 2(cig), 961]
    sr = np.zeros((NCORES, BPC, 128, 2, 968), f32)
    sr[..., :961] = search.reshape(NCORES, BPC, 2, 128, 961).transpose(0, 1, 3, 2, 4)
    # kernel [64,256,7,7] -> per core [128(ci), 2(cig), 8(s), 49]
    kr = np.zeros((NCORES, 128, 2, BPC, 52), f32)
    kr[..., :49] = kern.reshape(NCORES, BPC, 2, 128, 49).transpose(0, 3, 2, 1, 4)

    in_maps = []
    for c in range(NCORES):
        in_maps.append({
            "search": np.ascontiguousarray(sr[c]),
            "tmpl": np.ascontiguousarray(kr[c]),
            "wkt": wkt, "wst": wst, "w1t": w1t, "w2t": w2t,
            "bnk": bnk, "bns": bns, "bnh": bnh, "b2t": b2t, "ident": ident,
        })
    return in_maps


def get_program(bench_R=0):
    key = f"nc{bench_R}"
    if key not in _CACHE:
        _CACHE[key] = _build(bench_R)
    return _CACHE[key]


def kernel(**inputs):
    from concourse.bass_utils import run_bass_kernel_spmd
    nc = get_program()
    in_maps = _pack(inputs)
    res = run_bass_kernel_spmd(nc, in_maps, core_ids=list(range(NCORES)))
    out = np.stack([res.results[c]["out"] for c in range(NCORES)], axis=0)
    return out.reshape(64, 20, 25, 25).astype(np.float32)

